# revision 12
# baseline (speedup 1.0000x reference)
# Trainium2 Bass kernel for nn_CrossAttention (B=1, I=J=1024, C_S=1024,
# C_Z=128, H=16, D=64), sharded over the query dim i across 8 NeuronCores.
#
# v4: all large inputs are pre-cast (bias to fp8-e4m3, rest to bf16) AND
# pre-laid-out on the host so every device DMA is a plain contiguous
# [128, N] load (128 big descriptors, full HBM rate, no on-chip transposes):
#   - weights arrive as W^T in [p, co, f] chunk layout; s, k_in transposed
#     likewise; bias arrives as bias^T [c, i, j] per core in e4m3 (the z
#     pair-bias projection tolerates fp8: measured out-relerr 0.006 vs the
#     f64 reference, on top of ~0.005 from the bf16 pipeline).
#   - z[j, i, h] = bias^T Wz via per-(i, jc) stationary matmuls (N=16,
#     fp8 stationary x bf16 moving), interleaved with the projections.
#   - attention is j-major: scores [j, i] accumulate on a DVE-prefilled
#     z slice in PSUM (4 key-chunks per 512-wide bank), one exp per bank,
#     exp output feeds the o-matmul directly as the stationary operand.
#     Softmax denominator comes from an extra masked ones-column on v.
#
# kernel(**inputs) takes FULL inputs, shards on host, runs SPMD on cores 0-7,
# gathers to the full [1, 1024, 1024] output.

import numpy as np

B, I, J, CS, CZ, H, D = 1, 1024, 1024, 1024, 128, 16, 64
NCORES = 8
NI = I // NCORES  # 128 query rows per core
P = 128
KC = CS // P  # 8 contraction chunks
JC = J // P  # 8 key chunks
IC8 = 4  # i rows per bias chunk
NCHUNK = NI // IC8  # 32 bias chunks

_last_results = None


def _build_program():
    from contextlib import ExitStack

    import concourse.mybir as mybir
    import concourse.tile as tile
    from concourse import bacc
    from concourse.masks import make_identity

    f32 = mybir.dt.float32
    bf16 = mybir.dt.bfloat16
    fp8 = mybir.dt.float8e4
    AF = mybir.ActivationFunctionType
    ALU = mybir.AluOpType

    nc = bacc.Bacc("TRN2", target_bir_lowering=False, debug=False)

    # ---- dram io (host-prepared layouts, all partition-major) ----
    sT_d = nc.dram_tensor("sT", [P, KC, NI], bf16, kind="ExternalInput").ap()
    kinT_d = nc.dram_tensor("kinT", [P, KC, J], bf16, kind="ExternalInput").ap()
    biasT_d = nc.dram_tensor("biasT", [P, NI, J], fp8, kind="ExternalInput").ap()
    wqT_d = nc.dram_tensor("wqT", [P, KC, CS], bf16, kind="ExternalInput").ap()
    wkT_d = nc.dram_tensor("wkT", [P, KC, CS], bf16, kind="ExternalInput").ap()
    wvT_d = nc.dram_tensor("wvT", [P, KC, CS], bf16, kind="ExternalInput").ap()
    wgT_d = nc.dram_tensor("wgT", [P, KC, CS], bf16, kind="ExternalInput").ap()
    woT_d = nc.dram_tensor("woT", [P, KC, CS], bf16, kind="ExternalInput").ap()
    wz_d = nc.dram_tensor("w_z", [CZ, H], bf16, kind="ExternalInput").ap()
    bq_d = nc.dram_tensor("b_q", [P, KC], f32, kind="ExternalInput").ap()
    mask_d = nc.dram_tensor("mask", [P, JC], f32, kind="ExternalInput").ap()
    out_d = nc.dram_tensor("out", [NI, CS], f32, kind="ExternalOutput").ap()

    with tile.TileContext(nc) as tc, ExitStack() as ctx:
        pool = lambda name, bufs: ctx.enter_context(tc.tile_pool(name=name, bufs=bufs))
        ppool = lambda name, bufs: ctx.enter_context(
            tc.tile_pool(name=name, bufs=bufs, space="PSUM")
        )

        const = pool("const", 1)
        act_p = pool("act", 1)  # persistent small activations
        big_p = pool("big", 1)  # persistent big tensors (kinT, kT, v, z)
        bstage_p = pool("bstage", 3)  # bias^T chunks
        wstage_p = pool("wstage", 2)  # weight chunks
        et_p = pool("et", 3)
        st_p = pool("st", 2)
        outs_p = pool("outs", 2)

        big_ps = ppool("bigps", 2)  # [128,512] f32: projections / o-proj / go-T
        zq_ps = ppool("zqps", 4)  # [128,512] f32: z accumulation, then qk banks
        op_ps = ppool("ops", 2)  # [128,65] f32: o accumulators

        def copy_on(eng_is_vector, out, in_):
            if eng_is_vector:
                nc.vector.tensor_copy(out, in_)
            else:
                nc.scalar.copy(out, in_)

        # ---- constants / small loads (sync ring) ----
        ident = const.tile([P, P], bf16)
        make_identity(nc, ident)
        wz_s = const.tile([CZ, H], bf16)
        nc.sync.dma_start(wz_s, wz_d)
        gate_s = const.tile([1, 8], f32, tag="gate")

        def gate(dst_probe, src_probe):
            # tiny read of src+dst: delays dst's upcoming DMA (WAR) until
            # src_probe's producer has landed, serializing HBM traffic to
            # match PE consumption order
            nc.vector.tensor_tensor(gate_s[0:1, 0:1], src_probe, dst_probe, ALU.add)

        def load_w(w_ap, tag, gate_src=None):
            w = wstage_p.tile([P, KC, CS], bf16, tag="w", name=tag)
            if gate_src is not None:
                gate(w[0:1, 0, 0:1], gate_src)
            nc.scalar.dma_start(w, w_ap)
            return w

        # ---- z: bias^T chunks (plain DMA) + per-(i, jc) matmuls ----
        # z_s layout: [j_part, jc, h, i] (bf16) -- i contiguous for the
        # score adds in the attention inner loop
        z_s = big_p.tile([P, JC, H, NI], bf16, tag="z")

        def z_chunk(ic):
            bt = bstage_p.tile([P, IC8, J], fp8, tag="bt", name=f"bt_{ic}")
            nc.sync.dma_start(bt, biasT_d[:, ic * IC8 : (ic + 1) * IC8, :])
            # all 8 jc in one psum bank: [j=128, (8 jc, 4 i, 16 h)=512]
            zp = zq_ps.tile([P, 512], f32, tag="zq", name=f"zp_{ic}")
            for jc in range(JC):
                for il in range(IC8):
                    nc.tensor.matmul(
                        zp[:, jc * 64 + il * H : jc * 64 + (il + 1) * H],
                        bt[:, il, jc * P : (jc + 1) * P],
                        wz_s,
                        start=True,
                        stop=True,
                    )
            copy_on(
                ic % 2 == 0,
                z_s[:, :, :, ic * IC8 : (ic + 1) * IC8],
                zp.rearrange("p (a b c) -> p a c b", a=JC, b=IC8),
            )

        # ---- q projection: qT [f, i] = Wq s^T (+bq, /sqrt(D)) ----
        wq_s = load_w(wqT_d, "wq")  # first on the scalar ring: unlocks q_proj
        qT_s = act_p.tile([P, KC, NI], bf16, tag="qT")

        def q_proj():
            for fh in range(2):
                ps = big_ps.tile([P, 512], f32, tag="big", name=f"qp_{fh}")
                for fol in range(4):
                    fo = fh * 4 + fol
                    for co in range(KC):
                        nc.tensor.matmul(
                            ps[:, fol * P : (fol + 1) * P],
                            wq_s[:, co, fo * P : (fo + 1) * P],
                            sT_s[:, co, :],
                            start=(co == 0),
                            stop=(co == KC - 1),
                        )
                for fol in range(4):
                    fo = fh * 4 + fol
                    nc.vector.tensor_scalar(
                        qT_s[:, fo, :],
                        ps[:, fol * P : (fol + 1) * P],
                        bq_s[:, fo : fo + 1],
                        1.0 / np.sqrt(D),
                        ALU.add,
                        ALU.mult,
                    )

        z_chunk(0)
        z_chunk(1)
        z_chunk(2)
        sT_s = act_p.tile([P, KC, NI], bf16, tag="sT")
        nc.sync.dma_start(sT_s, sT_d)
        kinT_s = big_p.tile([P, KC, J], bf16, tag="kinT")
        nc.sync.dma_start(kinT_s, kinT_d)
        bq_s = const.tile([P, KC], f32)
        nc.sync.dma_start(bq_s, bq_d)
        mask_s = const.tile([P, JC], f32)
        nc.sync.dma_start(mask_s, mask_d)
        z_chunk(3)
        q_proj()
        z_chunk(4)
        z_chunk(5)

        # ---- k projection: kT [f, j] = Wk k_in^T ----
        wk_s = load_w(wkT_d, "wk", gate_src=z_s[0:1, 0, 0, 4:5])
        kT_s = big_p.tile([P, KC, J], bf16, tag="kT")
        for fo in range(KC):
            for jh in range(2):
                ps = big_ps.tile([P, 512], f32, tag="big", name=f"kp_{fo}_{jh}")
                for co in range(KC):
                    nc.tensor.matmul(
                        ps,
                        wk_s[:, co, fo * P : (fo + 1) * P],
                        kinT_s[:, co, jh * 512 : (jh + 1) * 512],
                        start=(co == 0),
                        stop=(co == KC - 1),
                    )
                copy_on(jh == 0, kT_s[:, fo, jh * 512 : (jh + 1) * 512], ps)
            z_chunk(6 + fo)

        # ---- v projection: v [j, h, d|mask] = k_in Wv^T, masked ----
        wv_s = load_w(wvT_d, "wv")
        v_s = big_p.tile([P, JC, H, D + 1], bf16, tag="v")
        for jo in range(JC):
            for fh in range(2):
                ps = big_ps.tile([P, 512], f32, tag="big", name=f"vp_{jo}_{fh}")
                for co in range(KC):
                    nc.tensor.matmul(
                        ps,
                        kinT_s[:, co, jo * P : (jo + 1) * P],
                        wv_s[:, co, fh * 512 : (fh + 1) * 512],
                        start=(co == 0),
                        stop=(co == KC - 1),
                    )
                nc.vector.tensor_scalar_mul(
                    v_s[:, jo, fh * 8 : (fh + 1) * 8, 0:D],
                    ps,
                    mask_s[:, jo : jo + 1],
                )
            nc.vector.tensor_copy(
                v_s[:, jo, :, D : D + 1],
                mask_s[:, jo : jo + 1, None].to_broadcast((P, H, 1)),
            )
            z_chunk(14 + jo)

        # ---- g projection: g [i, f] = sigmoid(s Wg^T) ----
        wg_s = load_w(wgT_d, "wg")
        g_s = act_p.tile([P, CS], bf16, tag="g")
        for fh in range(2):
            ps = big_ps.tile([P, 512], f32, tag="big", name=f"gp_{fh}")
            for co in range(KC):
                nc.tensor.matmul(
                    ps,
                    sT_s[:, co, :],
                    wg_s[:, co, fh * 512 : (fh + 1) * 512],
                    start=(co == 0),
                    stop=(co == KC - 1),
                )
            nc.scalar.activation(g_s[:, fh * 512 : (fh + 1) * 512], ps, AF.Sigmoid)
            z_chunk(22 + 2 * fh)
            z_chunk(23 + 2 * fh)

        wo_s = load_w(woT_d, "wo")
        for ic in range(26, NCHUNK):
            z_chunk(ic)

        # ---- attention: j-major scores, 4 key-chunks per 512-wide bank ----
        o_s = act_p.tile([P, CS], bf16, tag="o")
        goT = act_p.tile([P, KC, NI], bf16, tag="goT")
        for h in range(H):
            fo, pb = h // 2, (h % 2) * D
            op = op_ps.tile([P, D + 1], f32, tag="op", name=f"op_{h}")
            for jh in range(2):
                qk = zq_ps.tile([P, 512], f32, tag="zq", name=f"qk_{h}_{jh}")
                for jcl in range(4):
                    jc = jh * 4 + jcl
                    nc.tensor.matmul(
                        qk[:, jcl * P : (jcl + 1) * P],
                        kT_s[pb : pb + D, fo, jc * P : (jc + 1) * P],
                        qT_s[pb : pb + D, fo, :],
                        start=True,
                        stop=True,
                    )
                st = st_p.tile([P, 4, P], f32, tag="st", name=f"st_{h}_{jh}")
                nc.vector.tensor_tensor(
                    st,
                    qk.rearrange("p (a b) -> p a b", a=4),
                    z_s[:, jh * 4 : (jh + 1) * 4, h, :],
                    ALU.add,
                )
                et = et_p.tile([P, 512], bf16, tag="et", name=f"et_{h}_{jh}")
                nc.scalar.activation(et, st, AF.Exp)
                for jcl in range(4):
                    jc = jh * 4 + jcl
                    nc.tensor.matmul(
                        op,
                        et[:, jcl * P : (jcl + 1) * P],
                        v_s[:, jc, h, :],
                        start=(jc == 0),
                        stop=(jc == JC - 1),
                    )
            rec = et_p.tile([P, 1], f32, tag="rec", name=f"rec_{h}")
            nc.vector.reciprocal(rec, op[:, D : D + 1])
            nc.vector.tensor_scalar_mul(o_s[:, h * D : (h + 1) * D], op[:, 0:D], rec)
            if h % 8 == 7:
                # gate + transpose this half while later heads proceed
                gh = h // 8
                nc.vector.tensor_mul(
                    g_s[:, gh * 512 : (gh + 1) * 512],
                    g_s[:, gh * 512 : (gh + 1) * 512],
                    o_s[:, gh * 512 : (gh + 1) * 512],
                )
                tb = big_ps.tile([P, 512], bf16, tag="big", name=f"tb_{gh}")
                for fo in range(gh * 4, gh * 4 + 4):
                    nc.tensor.transpose(
                        tb[:, (fo % 4) * P : (fo % 4 + 1) * P],
                        g_s[:, fo * P : (fo + 1) * P],
                        ident,
                    )
                nc.vector.tensor_copy(goT[:, gh * 4 : (gh + 1) * 4, :], tb)

        for fh in range(2):
            ps = big_ps.tile([P, 512], f32, tag="big", name=f"op_ps_{fh}")
            for fo in range(KC):
                nc.tensor.matmul(
                    ps,
                    goT[:, fo, :],
                    wo_s[:, fo, fh * 512 : (fh + 1) * 512],
                    start=(fo == 0),
                    stop=(fo == KC - 1),
                )
            out_s = outs_p.tile([P, 512], f32, tag="outs", name=f"out_s{fh}")
            nc.scalar.copy(out_s, ps)
            nc.sync.dma_start(out_d[:, fh * 512 : (fh + 1) * 512], out_s)

    nc.compile()
    return nc


def _chunk128(a):
    # [n*128, m...] -> [128, n, m...] matching rearrange("(co p) m -> p co m")
    n = a.shape[0] // P
    return np.ascontiguousarray(a.reshape(n, P, -1).transpose(1, 0, 2))


def kernel(**inputs):
    global _last_results
    import ml_dtypes
    from concourse.bass_utils import run_bass_kernel_spmd

    bf = ml_dtypes.bfloat16
    s = np.asarray(inputs["s"], dtype=np.float32)[0]
    k_in = np.asarray(inputs["k_in"], dtype=np.float32)[0]
    mask = np.asarray(inputs["mask"], dtype=np.float32)[0]
    bias = np.asarray(inputs["bias"], dtype=np.float32)[0]
    bq = np.asarray(inputs["bq"], dtype=np.float32)
    mult = int(np.asarray(inputs.get("multiplicity", 1)))
    assert mult == 1, f"multiplicity={mult} not supported (B=1)"

    # host-side layout prep (cheap vs device HBM savings)
    sT = _chunk128(s.T.astype(bf))  # [p, co, i_full]
    kinT = _chunk128(k_in.T.astype(bf))  # [p, co, j]
    wT = {
        k: _chunk128(np.asarray(inputs[k], np.float32).T.astype(bf))
        for k in ("Wq", "Wk", "Wv", "Wg", "Wo")
    }
    wz = np.ascontiguousarray(np.asarray(inputs["Wz"], np.float32).astype(bf))
    bq_r = np.ascontiguousarray(bq.reshape(KC, P).T)  # [p, fo] f32
    mask_r = np.ascontiguousarray(mask.reshape(JC, P).T)  # [p, jo] f32
    bias_q = bias.astype(ml_dtypes.float8_e4m3)  # [i_full, j, c]

    nc = _build_program()

    in_maps = []
    for c in range(NCORES):
        # bias^T per core: [c=128, i=128, j=1024]
        biasT = np.ascontiguousarray(
            bias_q[c * NI : (c + 1) * NI].transpose(2, 0, 1)
        )
        in_maps.append(
            {
                "sT": np.ascontiguousarray(sT[:, :, c * NI : (c + 1) * NI]),
                "kinT": kinT,
                "biasT": biasT,
                "wqT": wT["Wq"],
                "wkT": wT["Wk"],
                "wvT": wT["Wv"],
                "wgT": wT["Wg"],
                "woT": wT["Wo"],
                "w_z": wz,
                "b_q": bq_r,
                "mask": mask_r,
            }
        )

    try:
        res = run_bass_kernel_spmd(nc, in_maps, core_ids=list(range(NCORES)))
    except Exception:
        # transient device-unrecoverable errors have been observed on a
        # first attempt; one retry has always succeeded
        import time as _time

        _time.sleep(5.0)
        res = run_bass_kernel_spmd(nc, in_maps, core_ids=list(range(NCORES)))
    _last_results = res
    out = np.concatenate([r["out"] for r in res.results], axis=0)
    return out.reshape(B, I, CS).astype(np.float32)


if __name__ == "__main__":
    rng = np.random.default_rng(0)
    ins = {
        "s": rng.standard_normal((B, I, CS), dtype=np.float32),
        "k_in": rng.standard_normal((B, J, CS), dtype=np.float32),
        "mask": np.ones((B, J), np.float32),
        "bias": rng.standard_normal((B, I, J, CZ), dtype=np.float32),
        "Wq": rng.standard_normal((CS, CS), dtype=np.float32) * 0.02,
        "bq": rng.standard_normal((CS,), dtype=np.float32) * 0.02,
        "Wk": rng.standard_normal((CS, CS), dtype=np.float32) * 0.02,
        "Wv": rng.standard_normal((CS, CS), dtype=np.float32) * 0.02,
        "Wg": rng.standard_normal((CS, CS), dtype=np.float32) * 0.02,
        "Wo": rng.standard_normal((CS, CS), dtype=np.float32) * 0.02,
        "Wz": rng.standard_normal((CZ, H), dtype=np.float32) * 0.02,
        "multiplicity": 1,
    }
    out = kernel(**ins)
    print(out.shape, out.dtype)


# revision 13
# speedup vs baseline: 1.0020x; 1.0020x over previous
# Trainium2 Bass kernel for nn_CrossAttention (B=1, I=J=1024, C_S=1024,
# C_Z=128, H=16, D=64), sharded over the query dim i across 8 NeuronCores.
#
# v4: all large inputs are pre-cast (bias to fp8-e4m3, rest to bf16) AND
# pre-laid-out on the host so every device DMA is a plain contiguous
# [128, N] load (128 big descriptors, full HBM rate, no on-chip transposes):
#   - weights arrive as W^T in [p, co, f] chunk layout; s, k_in transposed
#     likewise; bias arrives as bias^T [c, i, j] per core in e4m3 (the z
#     pair-bias projection tolerates fp8: measured out-relerr 0.006 vs the
#     f64 reference, on top of ~0.005 from the bf16 pipeline).
#   - z[j, i, h] = bias^T Wz via per-(i, jc) stationary matmuls (N=16,
#     fp8 stationary x bf16 moving), interleaved with the projections.
#   - attention is j-major: scores [j, i] accumulate on a DVE-prefilled
#     z slice in PSUM (4 key-chunks per 512-wide bank), one exp per bank,
#     exp output feeds the o-matmul directly as the stationary operand.
#     Softmax denominator comes from an extra masked ones-column on v.
#
# kernel(**inputs) takes FULL inputs, shards on host, runs SPMD on cores 0-7,
# gathers to the full [1, 1024, 1024] output.

import numpy as np

B, I, J, CS, CZ, H, D = 1, 1024, 1024, 1024, 128, 16, 64
NCORES = 8
NI = I // NCORES  # 128 query rows per core
P = 128
KC = CS // P  # 8 contraction chunks
JC = J // P  # 8 key chunks
IC8 = 4  # i rows per bias chunk
NCHUNK = NI // IC8  # 32 bias chunks

_last_results = None


def _build_program():
    from contextlib import ExitStack

    import concourse.mybir as mybir
    import concourse.tile as tile
    from concourse import bacc
    from concourse.masks import make_identity

    f32 = mybir.dt.float32
    bf16 = mybir.dt.bfloat16
    fp8 = mybir.dt.float8e4
    AF = mybir.ActivationFunctionType
    ALU = mybir.AluOpType

    nc = bacc.Bacc("TRN2", target_bir_lowering=False, debug=False)

    # ---- dram io (host-prepared layouts, all partition-major) ----
    sT_d = nc.dram_tensor("sT", [P, KC, NI], bf16, kind="ExternalInput").ap()
    kinT_d = nc.dram_tensor("kinT", [P, KC, J], bf16, kind="ExternalInput").ap()
    biasT_d = nc.dram_tensor("biasT", [P, NI, J], fp8, kind="ExternalInput").ap()
    wqT_d = nc.dram_tensor("wqT", [P, KC, CS], bf16, kind="ExternalInput").ap()
    wkT_d = nc.dram_tensor("wkT", [P, KC, CS], bf16, kind="ExternalInput").ap()
    wvT_d = nc.dram_tensor("wvT", [P, KC, CS], bf16, kind="ExternalInput").ap()
    wgT_d = nc.dram_tensor("wgT", [P, KC, CS], bf16, kind="ExternalInput").ap()
    woT_d = nc.dram_tensor("woT", [P, KC, CS], bf16, kind="ExternalInput").ap()
    wz_d = nc.dram_tensor("w_z", [CZ, H], bf16, kind="ExternalInput").ap()
    bq_d = nc.dram_tensor("b_q", [P, KC], f32, kind="ExternalInput").ap()
    mask_d = nc.dram_tensor("mask", [P, JC], f32, kind="ExternalInput").ap()
    out_d = nc.dram_tensor("out", [NI, CS], f32, kind="ExternalOutput").ap()

    with tile.TileContext(nc) as tc, ExitStack() as ctx:
        pool = lambda name, bufs: ctx.enter_context(tc.tile_pool(name=name, bufs=bufs))
        ppool = lambda name, bufs: ctx.enter_context(
            tc.tile_pool(name=name, bufs=bufs, space="PSUM")
        )

        const = pool("const", 1)
        act_p = pool("act", 1)  # persistent small activations
        big_p = pool("big", 1)  # persistent big tensors (kinT, kT, v, z)
        bstage_p = pool("bstage", 4)  # bias^T chunks
        wstage_p = pool("wstage", 2)  # weight chunks
        et_p = pool("et", 3)
        st_p = pool("st", 2)
        outs_p = pool("outs", 2)

        big_ps = ppool("bigps", 2)  # [128,512] f32: projections / o-proj / go-T
        zq_ps = ppool("zqps", 4)  # [128,512] f32: z accumulation, then qk banks
        op_ps = ppool("ops", 2)  # [128,65] f32: o accumulators

        def copy_on(eng_is_vector, out, in_):
            if eng_is_vector:
                nc.vector.tensor_copy(out, in_)
            else:
                nc.scalar.copy(out, in_)

        # ---- constants / small loads (sync ring) ----
        ident = const.tile([P, P], bf16)
        make_identity(nc, ident)
        wz_s = const.tile([CZ, H], bf16)
        nc.sync.dma_start(wz_s, wz_d)

        def load_w(w_ap, tag):
            w = wstage_p.tile([P, KC, CS], bf16, tag="w", name=tag)
            nc.scalar.dma_start(w, w_ap)
            return w

        # ---- z: bias^T chunks (plain DMA) + per-(i, jc) matmuls ----
        # z_s layout: [j_part, jc, h, i] (bf16) -- i contiguous for the
        # score adds in the attention inner loop
        z_s = big_p.tile([P, JC, H, NI], bf16, tag="z")

        def z_chunk(ic):
            bt = bstage_p.tile([P, IC8, J], fp8, tag="bt", name=f"bt_{ic}")
            nc.sync.dma_start(bt, biasT_d[:, ic * IC8 : (ic + 1) * IC8, :])
            # all 8 jc in one psum bank: [j=128, (8 jc, 4 i, 16 h)=512]
            zp = zq_ps.tile([P, 512], f32, tag="zq", name=f"zp_{ic}")
            for jc in range(JC):
                for il in range(IC8):
                    nc.tensor.matmul(
                        zp[:, jc * 64 + il * H : jc * 64 + (il + 1) * H],
                        bt[:, il, jc * P : (jc + 1) * P],
                        wz_s,
                        start=True,
                        stop=True,
                    )
            nc.vector.tensor_copy(
                z_s[:, :, :, ic * IC8 : (ic + 1) * IC8],
                zp.rearrange("p (a b c) -> p a c b", a=JC, b=IC8),
            )

        # ---- q projection: qT [f, i] = Wq s^T (+bq, /sqrt(D)) ----
        wq_s = load_w(wqT_d, "wq")  # first on the scalar ring: unlocks q_proj
        qT_s = act_p.tile([P, KC, NI], bf16, tag="qT")

        def q_proj():
            for fh in range(2):
                ps = big_ps.tile([P, 512], f32, tag="big", name=f"qp_{fh}")
                for fol in range(4):
                    fo = fh * 4 + fol
                    for co in range(KC):
                        nc.tensor.matmul(
                            ps[:, fol * P : (fol + 1) * P],
                            wq_s[:, co, fo * P : (fo + 1) * P],
                            sT_s[:, co, :],
                            start=(co == 0),
                            stop=(co == KC - 1),
                        )
                for fol in range(4):
                    fo = fh * 4 + fol
                    nc.vector.tensor_scalar(
                        qT_s[:, fo, :],
                        ps[:, fol * P : (fol + 1) * P],
                        bq_s[:, fo : fo + 1],
                        1.0 / np.sqrt(D),
                        ALU.add,
                        ALU.mult,
                    )

        z_chunk(0)
        z_chunk(1)
        z_chunk(2)
        sT_s = act_p.tile([P, KC, NI], bf16, tag="sT")
        nc.sync.dma_start(sT_s, sT_d)
        kinT_s = big_p.tile([P, KC, J], bf16, tag="kinT")
        nc.sync.dma_start(kinT_s, kinT_d)
        bq_s = const.tile([P, KC], f32)
        nc.sync.dma_start(bq_s, bq_d)
        mask_s = const.tile([P, JC], f32)
        nc.sync.dma_start(mask_s, mask_d)
        z_chunk(3)
        q_proj()
        z_chunk(4)
        z_chunk(5)

        # ---- k projection: kT [f, j] = Wk k_in^T ----
        wk_s = load_w(wkT_d, "wk")
        kT_s = big_p.tile([P, KC, J], bf16, tag="kT")
        for fo in range(KC):
            for jh in range(2):
                ps = big_ps.tile([P, 512], f32, tag="big", name=f"kp_{fo}_{jh}")
                for co in range(KC):
                    nc.tensor.matmul(
                        ps,
                        wk_s[:, co, fo * P : (fo + 1) * P],
                        kinT_s[:, co, jh * 512 : (jh + 1) * 512],
                        start=(co == 0),
                        stop=(co == KC - 1),
                    )
                copy_on(jh == 0, kT_s[:, fo, jh * 512 : (jh + 1) * 512], ps)
            z_chunk(6 + fo)

        # ---- v projection: v [j, h, d|mask] = k_in Wv^T, masked ----
        wv_s = load_w(wvT_d, "wv")
        v_s = big_p.tile([P, JC, H, D + 1], bf16, tag="v")
        for jo in range(JC):
            for fh in range(2):
                ps = big_ps.tile([P, 512], f32, tag="big", name=f"vp_{jo}_{fh}")
                for co in range(KC):
                    nc.tensor.matmul(
                        ps,
                        kinT_s[:, co, jo * P : (jo + 1) * P],
                        wv_s[:, co, fh * 512 : (fh + 1) * 512],
                        start=(co == 0),
                        stop=(co == KC - 1),
                    )
                nc.vector.tensor_scalar_mul(
                    v_s[:, jo, fh * 8 : (fh + 1) * 8, 0:D],
                    ps,
                    mask_s[:, jo : jo + 1],
                )
            nc.vector.tensor_copy(
                v_s[:, jo, :, D : D + 1],
                mask_s[:, jo : jo + 1, None].to_broadcast((P, H, 1)),
            )
            z_chunk(14 + jo)

        # ---- g projection: g [i, f] = sigmoid(s Wg^T) ----
        wg_s = load_w(wgT_d, "wg")
        g_s = act_p.tile([P, CS], bf16, tag="g")
        for fh in range(2):
            ps = big_ps.tile([P, 512], f32, tag="big", name=f"gp_{fh}")
            for co in range(KC):
                nc.tensor.matmul(
                    ps,
                    sT_s[:, co, :],
                    wg_s[:, co, fh * 512 : (fh + 1) * 512],
                    start=(co == 0),
                    stop=(co == KC - 1),
                )
            nc.scalar.activation(g_s[:, fh * 512 : (fh + 1) * 512], ps, AF.Sigmoid)
            z_chunk(22 + 2 * fh)
            z_chunk(23 + 2 * fh)

        wo_s = load_w(woT_d, "wo")
        for ic in range(26, NCHUNK):
            z_chunk(ic)

        # ---- attention: j-major scores, 4 key-chunks per 512-wide bank ----
        o_s = act_p.tile([P, CS], bf16, tag="o")
        goT = act_p.tile([P, KC, NI], bf16, tag="goT")
        for h in range(H):
            fo, pb = h // 2, (h % 2) * D
            op = op_ps.tile([P, D + 1], f32, tag="op", name=f"op_{h}")
            for jh in range(2):
                qk = zq_ps.tile([P, 512], f32, tag="zq", name=f"qk_{h}_{jh}")
                for jcl in range(4):
                    jc = jh * 4 + jcl
                    nc.tensor.matmul(
                        qk[:, jcl * P : (jcl + 1) * P],
                        kT_s[pb : pb + D, fo, jc * P : (jc + 1) * P],
                        qT_s[pb : pb + D, fo, :],
                        start=True,
                        stop=True,
                    )
                st = st_p.tile([P, 4, P], f32, tag="st", name=f"st_{h}_{jh}")
                nc.vector.tensor_tensor(
                    st,
                    qk.rearrange("p (a b) -> p a b", a=4),
                    z_s[:, jh * 4 : (jh + 1) * 4, h, :],
                    ALU.add,
                )
                et = et_p.tile([P, 512], bf16, tag="et", name=f"et_{h}_{jh}")
                nc.scalar.activation(et, st, AF.Exp)
                for jcl in range(4):
                    jc = jh * 4 + jcl
                    nc.tensor.matmul(
                        op,
                        et[:, jcl * P : (jcl + 1) * P],
                        v_s[:, jc, h, :],
                        start=(jc == 0),
                        stop=(jc == JC - 1),
                    )
            rec = et_p.tile([P, 1], f32, tag="rec", name=f"rec_{h}")
            nc.vector.reciprocal(rec, op[:, D : D + 1])
            nc.vector.tensor_scalar_mul(o_s[:, h * D : (h + 1) * D], op[:, 0:D], rec)
            if h % 8 == 7:
                # gate + transpose this half while later heads proceed
                gh = h // 8
                nc.vector.tensor_mul(
                    g_s[:, gh * 512 : (gh + 1) * 512],
                    g_s[:, gh * 512 : (gh + 1) * 512],
                    o_s[:, gh * 512 : (gh + 1) * 512],
                )
                tb = big_ps.tile([P, 512], bf16, tag="big", name=f"tb_{gh}")
                for fo in range(gh * 4, gh * 4 + 4):
                    nc.tensor.transpose(
                        tb[:, (fo % 4) * P : (fo % 4 + 1) * P],
                        g_s[:, fo * P : (fo + 1) * P],
                        ident,
                    )
                nc.vector.tensor_copy(goT[:, gh * 4 : (gh + 1) * 4, :], tb)

        for fh in range(2):
            ps = big_ps.tile([P, 512], f32, tag="big", name=f"op_ps_{fh}")
            for fo in range(KC):
                nc.tensor.matmul(
                    ps,
                    goT[:, fo, :],
                    wo_s[:, fo, fh * 512 : (fh + 1) * 512],
                    start=(fo == 0),
                    stop=(fo == KC - 1),
                )
            out_s = outs_p.tile([P, 512], f32, tag="outs", name=f"out_s{fh}")
            nc.scalar.copy(out_s, ps)
            nc.sync.dma_start(out_d[:, fh * 512 : (fh + 1) * 512], out_s)

    nc.compile()
    return nc


def _chunk128(a):
    # [n*128, m...] -> [128, n, m...] matching rearrange("(co p) m -> p co m")
    n = a.shape[0] // P
    return np.ascontiguousarray(a.reshape(n, P, -1).transpose(1, 0, 2))


def kernel(**inputs):
    global _last_results
    import ml_dtypes
    from concourse.bass_utils import run_bass_kernel_spmd

    bf = ml_dtypes.bfloat16
    s = np.asarray(inputs["s"], dtype=np.float32)[0]
    k_in = np.asarray(inputs["k_in"], dtype=np.float32)[0]
    mask = np.asarray(inputs["mask"], dtype=np.float32)[0]
    bias = np.asarray(inputs["bias"], dtype=np.float32)[0]
    bq = np.asarray(inputs["bq"], dtype=np.float32)
    mult = int(np.asarray(inputs.get("multiplicity", 1)))
    assert mult == 1, f"multiplicity={mult} not supported (B=1)"

    # host-side layout prep (cheap vs device HBM savings)
    sT = _chunk128(s.T.astype(bf))  # [p, co, i_full]
    kinT = _chunk128(k_in.T.astype(bf))  # [p, co, j]
    wT = {
        k: _chunk128(np.asarray(inputs[k], np.float32).T.astype(bf))
        for k in ("Wq", "Wk", "Wv", "Wg", "Wo")
    }
    wz = np.ascontiguousarray(np.asarray(inputs["Wz"], np.float32).astype(bf))
    bq_r = np.ascontiguousarray(bq.reshape(KC, P).T)  # [p, fo] f32
    mask_r = np.ascontiguousarray(mask.reshape(JC, P).T)  # [p, jo] f32
    bias_q = bias.astype(ml_dtypes.float8_e4m3)  # [i_full, j, c]

    nc = _build_program()

    in_maps = []
    for c in range(NCORES):
        # bias^T per core: [c=128, i=128, j=1024]
        biasT = np.ascontiguousarray(
            bias_q[c * NI : (c + 1) * NI].transpose(2, 0, 1)
        )
        in_maps.append(
            {
                "sT": np.ascontiguousarray(sT[:, :, c * NI : (c + 1) * NI]),
                "kinT": kinT,
                "biasT": biasT,
                "wqT": wT["Wq"],
                "wkT": wT["Wk"],
                "wvT": wT["Wv"],
                "wgT": wT["Wg"],
                "woT": wT["Wo"],
                "w_z": wz,
                "b_q": bq_r,
                "mask": mask_r,
            }
        )

    try:
        res = run_bass_kernel_spmd(nc, in_maps, core_ids=list(range(NCORES)))
    except Exception:
        # transient device-unrecoverable errors have been observed on a
        # first attempt; one retry has always succeeded
        import time as _time

        _time.sleep(5.0)
        res = run_bass_kernel_spmd(nc, in_maps, core_ids=list(range(NCORES)))
    _last_results = res
    out = np.concatenate([r["out"] for r in res.results], axis=0)
    return out.reshape(B, I, CS).astype(np.float32)


if __name__ == "__main__":
    rng = np.random.default_rng(0)
    ins = {
        "s": rng.standard_normal((B, I, CS), dtype=np.float32),
        "k_in": rng.standard_normal((B, J, CS), dtype=np.float32),
        "mask": np.ones((B, J), np.float32),
        "bias": rng.standard_normal((B, I, J, CZ), dtype=np.float32),
        "Wq": rng.standard_normal((CS, CS), dtype=np.float32) * 0.02,
        "bq": rng.standard_normal((CS,), dtype=np.float32) * 0.02,
        "Wk": rng.standard_normal((CS, CS), dtype=np.float32) * 0.02,
        "Wv": rng.standard_normal((CS, CS), dtype=np.float32) * 0.02,
        "Wg": rng.standard_normal((CS, CS), dtype=np.float32) * 0.02,
        "Wo": rng.standard_normal((CS, CS), dtype=np.float32) * 0.02,
        "Wz": rng.standard_normal((CZ, H), dtype=np.float32) * 0.02,
        "multiplicity": 1,
    }
    out = kernel(**ins)
    print(out.shape, out.dtype)


# revision 14
# speedup vs baseline: 1.0186x; 1.0166x over previous
# Trainium2 Bass kernel for nn_CrossAttention (B=1, I=J=1024, C_S=1024,
# C_Z=128, H=16, D=64), sharded over the query dim i across 8 NeuronCores.
#
# v4: all large inputs are pre-cast (bias to fp8-e4m3, rest to bf16) AND
# pre-laid-out on the host so every device DMA is a plain contiguous
# [128, N] load (128 big descriptors, full HBM rate, no on-chip transposes):
#   - weights arrive as W^T in [p, co, f] chunk layout; s, k_in transposed
#     likewise; bias arrives as bias^T [c, i, j] per core in e4m3 (the z
#     pair-bias projection tolerates fp8: measured out-relerr 0.006 vs the
#     f64 reference, on top of ~0.005 from the bf16 pipeline).
#   - z[j, i, h] = bias^T Wz via per-(i, jc) stationary matmuls (N=16,
#     fp8 stationary x bf16 moving), interleaved with the projections.
#   - attention is j-major: scores [j, i] accumulate on a DVE-prefilled
#     z slice in PSUM (4 key-chunks per 512-wide bank), one exp per bank,
#     exp output feeds the o-matmul directly as the stationary operand.
#     Softmax denominator comes from an extra masked ones-column on v.
#
# kernel(**inputs) takes FULL inputs, shards on host, runs SPMD on cores 0-7,
# gathers to the full [1, 1024, 1024] output.

import numpy as np

B, I, J, CS, CZ, H, D = 1, 1024, 1024, 1024, 128, 16, 64
NCORES = 8
NI = I // NCORES  # 128 query rows per core
P = 128
KC = CS // P  # 8 contraction chunks
JC = J // P  # 8 key chunks
IC8 = 4  # i rows per bias chunk
NCHUNK = NI // IC8  # 32 bias chunks

_last_results = None


def _build_program():
    from contextlib import ExitStack

    import concourse.mybir as mybir
    import concourse.tile as tile
    from concourse import bacc
    from concourse.masks import make_identity

    f32 = mybir.dt.float32
    bf16 = mybir.dt.bfloat16
    fp8 = mybir.dt.float8e4
    AF = mybir.ActivationFunctionType
    ALU = mybir.AluOpType

    nc = bacc.Bacc("TRN2", target_bir_lowering=False, debug=False)

    # ---- dram io (host-prepared layouts, all partition-major) ----
    sT_d = nc.dram_tensor("sT", [P, KC, NI], bf16, kind="ExternalInput").ap()
    kinT_d = nc.dram_tensor("kinT", [P, KC, J], bf16, kind="ExternalInput").ap()
    biasT_d = nc.dram_tensor("biasT", [P, NI, J], fp8, kind="ExternalInput").ap()
    wqT_d = nc.dram_tensor("wqT", [P, KC, CS], bf16, kind="ExternalInput").ap()
    wkT_d = nc.dram_tensor("wkT", [P, KC, CS], bf16, kind="ExternalInput").ap()
    wvT_d = nc.dram_tensor("wvT", [P, KC, CS], bf16, kind="ExternalInput").ap()
    wgT_d = nc.dram_tensor("wgT", [P, KC, CS], bf16, kind="ExternalInput").ap()
    woT_d = nc.dram_tensor("woT", [P, KC, CS], bf16, kind="ExternalInput").ap()
    wz_d = nc.dram_tensor("w_z", [CZ, H], bf16, kind="ExternalInput").ap()
    bq_d = nc.dram_tensor("b_q", [P, KC], f32, kind="ExternalInput").ap()
    mask_d = nc.dram_tensor("mask", [P, JC], f32, kind="ExternalInput").ap()
    out_d = nc.dram_tensor("out", [NI, CS], f32, kind="ExternalOutput").ap()

    with tile.TileContext(nc) as tc, ExitStack() as ctx:
        pool = lambda name, bufs: ctx.enter_context(tc.tile_pool(name=name, bufs=bufs))
        ppool = lambda name, bufs: ctx.enter_context(
            tc.tile_pool(name=name, bufs=bufs, space="PSUM")
        )

        const = pool("const", 1)
        act_p = pool("act", 1)  # persistent small activations
        big_p = pool("big", 1)  # persistent big tensors (kinT, kT, v, z)
        bstage_p = pool("bstage", 4)  # bias^T chunks
        wstage_p = pool("wstage", 2)  # weight chunks
        et_p = pool("et", 3)
        st_p = pool("st", 2)
        outs_p = pool("outs", 2)

        big_ps = ppool("bigps", 2)  # [128,512] f32: projections / o-proj / go-T
        zq_ps = ppool("zqps", 4)  # [128,512] f32: z accumulation, then qk banks
        op_ps = ppool("ops", 2)  # [128,65] f32: o accumulators

        def copy_on(eng_is_vector, out, in_):
            if eng_is_vector:
                nc.vector.tensor_copy(out, in_)
            else:
                nc.scalar.copy(out, in_)

        # ---- constants / small loads (sync ring) ----
        ident = const.tile([P, P], bf16)
        make_identity(nc, ident)
        wz_s = const.tile([CZ, H], bf16)
        nc.sync.dma_start(wz_s, wz_d)
        gate_s = const.tile([1, 8], f32, tag="gate")

        def gate(dst_probe, src_probe):
            # tiny read of src+dst: delays dst's upcoming DMA (WAR) until
            # src_probe's producer lands, keeping early HBM bandwidth for
            # the bias chunks the PE needs first
            nc.vector.tensor_tensor(gate_s[0:1, 0:1], src_probe, dst_probe, ALU.add)

        def load_w(w_ap, tag, gate_src=None):
            w = wstage_p.tile([P, KC, CS], bf16, tag="w", name=tag)
            if gate_src is not None:
                gate(w[0:1, 0, 0:1], gate_src)
            nc.scalar.dma_start(w, w_ap)
            return w

        # ---- z: bias^T chunks (plain DMA) + per-(i, jc) matmuls ----
        # z_s layout: [j_part, jc, h, i] (bf16) -- i contiguous for the
        # score adds in the attention inner loop
        z_s = big_p.tile([P, JC, H, NI], bf16, tag="z")

        def z_chunk(ic):
            bt = bstage_p.tile([P, IC8, J], fp8, tag="bt", name=f"bt_{ic}")
            nc.sync.dma_start(bt, biasT_d[:, ic * IC8 : (ic + 1) * IC8, :])
            # all 8 jc in one psum bank: [j=128, (8 jc, 4 i, 16 h)=512]
            zp = zq_ps.tile([P, 512], f32, tag="zq", name=f"zp_{ic}")
            for jc in range(JC):
                for il in range(IC8):
                    nc.tensor.matmul(
                        zp[:, jc * 64 + il * H : jc * 64 + (il + 1) * H],
                        bt[:, il, jc * P : (jc + 1) * P],
                        wz_s,
                        start=True,
                        stop=True,
                    )
            nc.vector.tensor_copy(
                z_s[:, :, :, ic * IC8 : (ic + 1) * IC8],
                zp.rearrange("p (a b c) -> p a c b", a=JC, b=IC8),
            )

        # ---- q projection: qT [f, i] = Wq s^T (+bq, /sqrt(D)) ----
        wq_s = load_w(wqT_d, "wq")  # first on the scalar ring: unlocks q_proj
        qT_s = act_p.tile([P, KC, NI], bf16, tag="qT")

        def q_proj():
            for fh in range(2):
                ps = big_ps.tile([P, 512], f32, tag="big", name=f"qp_{fh}")
                for fol in range(4):
                    fo = fh * 4 + fol
                    for co in range(KC):
                        nc.tensor.matmul(
                            ps[:, fol * P : (fol + 1) * P],
                            wq_s[:, co, fo * P : (fo + 1) * P],
                            sT_s[:, co, :],
                            start=(co == 0),
                            stop=(co == KC - 1),
                        )
                for fol in range(4):
                    fo = fh * 4 + fol
                    nc.vector.tensor_scalar(
                        qT_s[:, fo, :],
                        ps[:, fol * P : (fol + 1) * P],
                        bq_s[:, fo : fo + 1],
                        1.0 / np.sqrt(D),
                        ALU.add,
                        ALU.mult,
                    )

        z_chunk(0)
        z_chunk(1)
        z_chunk(2)
        sT_s = act_p.tile([P, KC, NI], bf16, tag="sT")
        nc.sync.dma_start(sT_s, sT_d)
        kinT_s = big_p.tile([P, KC, J], bf16, tag="kinT")
        gate(kinT_s[0:1, 0, 0:1], z_s[0:1, 0, 0, 0:1])
        nc.sync.dma_start(kinT_s, kinT_d)
        bq_s = const.tile([P, KC], f32)
        nc.sync.dma_start(bq_s, bq_d)
        mask_s = const.tile([P, JC], f32)
        nc.sync.dma_start(mask_s, mask_d)
        z_chunk(3)
        q_proj()
        z_chunk(4)
        z_chunk(5)

        # ---- k projection: kT [f, j] = Wk k_in^T ----
        wk_s = load_w(wkT_d, "wk", gate_src=z_s[0:1, 0, 0, 4:5])
        kT_s = big_p.tile([P, KC, J], bf16, tag="kT")
        for fo in range(KC):
            for jh in range(2):
                ps = big_ps.tile([P, 512], f32, tag="big", name=f"kp_{fo}_{jh}")
                for co in range(KC):
                    nc.tensor.matmul(
                        ps,
                        wk_s[:, co, fo * P : (fo + 1) * P],
                        kinT_s[:, co, jh * 512 : (jh + 1) * 512],
                        start=(co == 0),
                        stop=(co == KC - 1),
                    )
                copy_on(jh == 0, kT_s[:, fo, jh * 512 : (jh + 1) * 512], ps)
            z_chunk(6 + fo)

        # ---- v projection: v [j, h, d|mask] = k_in Wv^T, masked ----
        wv_s = load_w(wvT_d, "wv")
        v_s = big_p.tile([P, JC, H, D + 1], bf16, tag="v")
        for jo in range(JC):
            for fh in range(2):
                ps = big_ps.tile([P, 512], f32, tag="big", name=f"vp_{jo}_{fh}")
                for co in range(KC):
                    nc.tensor.matmul(
                        ps,
                        kinT_s[:, co, jo * P : (jo + 1) * P],
                        wv_s[:, co, fh * 512 : (fh + 1) * 512],
                        start=(co == 0),
                        stop=(co == KC - 1),
                    )
                nc.vector.tensor_scalar_mul(
                    v_s[:, jo, fh * 8 : (fh + 1) * 8, 0:D],
                    ps,
                    mask_s[:, jo : jo + 1],
                )
            nc.vector.tensor_copy(
                v_s[:, jo, :, D : D + 1],
                mask_s[:, jo : jo + 1, None].to_broadcast((P, H, 1)),
            )
            z_chunk(14 + jo)

        # ---- g projection: g [i, f] = sigmoid(s Wg^T) ----
        wg_s = load_w(wgT_d, "wg")
        g_s = act_p.tile([P, CS], bf16, tag="g")
        for fh in range(2):
            ps = big_ps.tile([P, 512], f32, tag="big", name=f"gp_{fh}")
            for co in range(KC):
                nc.tensor.matmul(
                    ps,
                    sT_s[:, co, :],
                    wg_s[:, co, fh * 512 : (fh + 1) * 512],
                    start=(co == 0),
                    stop=(co == KC - 1),
                )
            nc.scalar.activation(g_s[:, fh * 512 : (fh + 1) * 512], ps, AF.Sigmoid)
            z_chunk(22 + 2 * fh)
            z_chunk(23 + 2 * fh)

        wo_s = load_w(woT_d, "wo")
        for ic in range(26, NCHUNK):
            z_chunk(ic)

        # ---- attention: j-major scores, 4 key-chunks per 512-wide bank ----
        o_s = act_p.tile([P, CS], bf16, tag="o")
        goT = act_p.tile([P, KC, NI], bf16, tag="goT")
        for h in range(H):
            fo, pb = h // 2, (h % 2) * D
            op = op_ps.tile([P, D + 1], f32, tag="op", name=f"op_{h}")
            for jh in range(2):
                qk = zq_ps.tile([P, 512], f32, tag="zq", name=f"qk_{h}_{jh}")
                for jcl in range(4):
                    jc = jh * 4 + jcl
                    nc.tensor.matmul(
                        qk[:, jcl * P : (jcl + 1) * P],
                        kT_s[pb : pb + D, fo, jc * P : (jc + 1) * P],
                        qT_s[pb : pb + D, fo, :],
                        start=True,
                        stop=True,
                    )
                st = st_p.tile([P, 4, P], f32, tag="st", name=f"st_{h}_{jh}")
                nc.vector.tensor_tensor(
                    st,
                    qk.rearrange("p (a b) -> p a b", a=4),
                    z_s[:, jh * 4 : (jh + 1) * 4, h, :],
                    ALU.add,
                )
                et = et_p.tile([P, 512], bf16, tag="et", name=f"et_{h}_{jh}")
                nc.scalar.activation(et, st, AF.Exp)
                for jcl in range(4):
                    jc = jh * 4 + jcl
                    nc.tensor.matmul(
                        op,
                        et[:, jcl * P : (jcl + 1) * P],
                        v_s[:, jc, h, :],
                        start=(jc == 0),
                        stop=(jc == JC - 1),
                    )
            rec = et_p.tile([P, 1], f32, tag="rec", name=f"rec_{h}")
            nc.vector.reciprocal(rec, op[:, D : D + 1])
            nc.vector.tensor_scalar_mul(o_s[:, h * D : (h + 1) * D], op[:, 0:D], rec)
            if h % 8 == 7:
                # gate + transpose this half while later heads proceed
                gh = h // 8
                nc.vector.tensor_mul(
                    g_s[:, gh * 512 : (gh + 1) * 512],
                    g_s[:, gh * 512 : (gh + 1) * 512],
                    o_s[:, gh * 512 : (gh + 1) * 512],
                )
                tb = big_ps.tile([P, 512], bf16, tag="big", name=f"tb_{gh}")
                for fo in range(gh * 4, gh * 4 + 4):
                    nc.tensor.transpose(
                        tb[:, (fo % 4) * P : (fo % 4 + 1) * P],
                        g_s[:, fo * P : (fo + 1) * P],
                        ident,
                    )
                nc.vector.tensor_copy(goT[:, gh * 4 : (gh + 1) * 4, :], tb)

        for fh in range(2):
            ps = big_ps.tile([P, 512], f32, tag="big", name=f"op_ps_{fh}")
            for fo in range(KC):
                nc.tensor.matmul(
                    ps,
                    goT[:, fo, :],
                    wo_s[:, fo, fh * 512 : (fh + 1) * 512],
                    start=(fo == 0),
                    stop=(fo == KC - 1),
                )
            out_s = outs_p.tile([P, 512], f32, tag="outs", name=f"out_s{fh}")
            nc.scalar.copy(out_s, ps)
            nc.sync.dma_start(out_d[:, fh * 512 : (fh + 1) * 512], out_s)

    nc.compile()
    return nc


def _chunk128(a):
    # [n*128, m...] -> [128, n, m...] matching rearrange("(co p) m -> p co m")
    n = a.shape[0] // P
    return np.ascontiguousarray(a.reshape(n, P, -1).transpose(1, 0, 2))


def kernel(**inputs):
    global _last_results
    import ml_dtypes
    from concourse.bass_utils import run_bass_kernel_spmd

    bf = ml_dtypes.bfloat16
    s = np.asarray(inputs["s"], dtype=np.float32)[0]
    k_in = np.asarray(inputs["k_in"], dtype=np.float32)[0]
    mask = np.asarray(inputs["mask"], dtype=np.float32)[0]
    bias = np.asarray(inputs["bias"], dtype=np.float32)[0]
    bq = np.asarray(inputs["bq"], dtype=np.float32)
    mult = int(np.asarray(inputs.get("multiplicity", 1)))
    assert mult == 1, f"multiplicity={mult} not supported (B=1)"

    # host-side layout prep (cheap vs device HBM savings)
    sT = _chunk128(s.T.astype(bf))  # [p, co, i_full]
    kinT = _chunk128(k_in.T.astype(bf))  # [p, co, j]
    wT = {
        k: _chunk128(np.asarray(inputs[k], np.float32).T.astype(bf))
        for k in ("Wq", "Wk", "Wv", "Wg", "Wo")
    }
    wz = np.ascontiguousarray(np.asarray(inputs["Wz"], np.float32).astype(bf))
    bq_r = np.ascontiguousarray(bq.reshape(KC, P).T)  # [p, fo] f32
    mask_r = np.ascontiguousarray(mask.reshape(JC, P).T)  # [p, jo] f32
    bias_q = bias.astype(ml_dtypes.float8_e4m3)  # [i_full, j, c]

    nc = _build_program()

    in_maps = []
    for c in range(NCORES):
        # bias^T per core: [c=128, i=128, j=1024]
        biasT = np.ascontiguousarray(
            bias_q[c * NI : (c + 1) * NI].transpose(2, 0, 1)
        )
        in_maps.append(
            {
                "sT": np.ascontiguousarray(sT[:, :, c * NI : (c + 1) * NI]),
                "kinT": kinT,
                "biasT": biasT,
                "wqT": wT["Wq"],
                "wkT": wT["Wk"],
                "wvT": wT["Wv"],
                "wgT": wT["Wg"],
                "woT": wT["Wo"],
                "w_z": wz,
                "b_q": bq_r,
                "mask": mask_r,
            }
        )

    try:
        res = run_bass_kernel_spmd(nc, in_maps, core_ids=list(range(NCORES)))
    except Exception:
        # transient device-unrecoverable errors have been observed on a
        # first attempt; one retry has always succeeded
        import time as _time

        _time.sleep(5.0)
        res = run_bass_kernel_spmd(nc, in_maps, core_ids=list(range(NCORES)))
    _last_results = res
    out = np.concatenate([r["out"] for r in res.results], axis=0)
    return out.reshape(B, I, CS).astype(np.float32)


if __name__ == "__main__":
    rng = np.random.default_rng(0)
    ins = {
        "s": rng.standard_normal((B, I, CS), dtype=np.float32),
        "k_in": rng.standard_normal((B, J, CS), dtype=np.float32),
        "mask": np.ones((B, J), np.float32),
        "bias": rng.standard_normal((B, I, J, CZ), dtype=np.float32),
        "Wq": rng.standard_normal((CS, CS), dtype=np.float32) * 0.02,
        "bq": rng.standard_normal((CS,), dtype=np.float32) * 0.02,
        "Wk": rng.standard_normal((CS, CS), dtype=np.float32) * 0.02,
        "Wv": rng.standard_normal((CS, CS), dtype=np.float32) * 0.02,
        "Wg": rng.standard_normal((CS, CS), dtype=np.float32) * 0.02,
        "Wo": rng.standard_normal((CS, CS), dtype=np.float32) * 0.02,
        "Wz": rng.standard_normal((CZ, H), dtype=np.float32) * 0.02,
        "multiplicity": 1,
    }
    out = kernel(**ins)
    print(out.shape, out.dtype)
